# revision 11
# baseline (speedup 1.0000x reference)
"""Causal single-head attention (S=8192, D=512, fp32 I/O) on 8 TRN2 NeuronCores.

Strategy (sequence parallelism, causal-balanced, fully static SPMD graph):
- 16 row-blocks of 512 rows; core c owns blocks {c, 15-c} -> every core
  processes exactly 17 (block, key-tile) pairs of the causal lower triangle.
- The host hands each core a permuted+duplicated key layout xTs[512, 17*512]
  whose column-tile i holds the keys of the core's slot i, so all SBUF
  addressing is compile-time static. Slots 0/1 are statically the two
  diagonal tiles (triangular masks applied there only).
- Which of the core's two row-blocks a slot belongs to is data: per-slot 0/1
  selection scalars route the q window, the PV accumulation and the softmax
  row-sums with plain arithmetic (no registers, no dynamic APs).
- k/v projections are computed per core over the 17 key tiles (~6% duplicate
  work); q only for the core's 1024 rows. QK is computed in both layouts so
  softmax reduces along the free axis and PV needs no transposes.
- exp(scores) goes to a bf16 DRAM scratch; once row-sums complete, a
  normalize pass rescales by 1/l and writes the attention weights (bf16,
  host casts to f32). PV output is rescaled on-chip and stored f32.
"""
import functools
import numpy as np
import ml_dtypes

import concourse.bass as bass
import concourse.mybir as mybir
from concourse import tile
from concourse.bass_utils import run_bass_kernel_spmd

F32 = mybir.dt.float32
BF16 = mybir.dt.bfloat16
AF = mybir.ActivationFunctionType
ALU = mybir.AluOpType

S = 8192
D = 512
NB = 16          # row blocks
BLK = S // NB    # 512
NCORES = 8
SLOTS = NB + 1   # 17 (block, key-tile) pairs per core
KW = SLOTS * 512  # 8704 permuted key columns
SCALE = 1.0 / np.sqrt(np.float32(D))
NEG = -1.0e9


def core_blocks(c):
    return (c, NB - 1 - c)


def core_slots(c):
    """Slot i -> (block_sel, key_tile). Slots 0/1 are the diagonals."""
    b0, b1 = core_blocks(c)
    slots = [(0, b0), (1, b1)]
    slots += [(0, t) for t in range(b0)]
    slots += [(1, t) for t in range(b1)]
    assert len(slots) == SLOTS
    return slots


def split_waits(nc, limit=1):
    """This toolchain's codegen allows one sync-wait per instruction; hoist
    extras onto preceding NoOps on the same engine."""
    n_split = 0
    for fn in nc.m.functions:
        for bb in fn.blocks:
            out = []
            for inst in bb.instructions:
                si = inst.sync_info
                waits = list(si.on_wait) if si and si.on_wait else []
                if len(waits) > limit:
                    extra, keep = waits[:-limit], waits[-limit:]
                    while extra:
                        chunk, extra = extra[:limit], extra[limit:]
                        nop = mybir.InstNoOp(
                            name=inst.name + f"_wsplit{n_split}",
                            engine=inst.engine, ins=[], outs=[])
                        nop.sync_info = mybir.SyncInfo(on_wait=chunk, on_update=[])
                        out.append(nop)
                        n_split += 1
                    inst.sync_info = mybir.SyncInfo(
                        on_wait=keep, on_update=list(si.on_update or []))
                out.append(inst)
            bb.instructions[:] = out
    return n_split


def build():
    nc = bass.Bass()

    xTs_ext = nc.declare_dram_parameter("xTs", [D, KW], BF16, isOutput=False)
    xqT_ext = nc.declare_dram_parameter("xqT", [D, 2 * BLK], BF16, isOutput=False)
    wqT_ext = nc.declare_dram_parameter("wqT", [D, D], BF16, isOutput=False)
    wkT_ext = nc.declare_dram_parameter("wkT", [D, D], BF16, isOutput=False)
    wvT_ext = nc.declare_dram_parameter("wvT", [D, D], BF16, isOutput=False)
    bqk_ext = nc.declare_dram_parameter("bqk", [128, 8], F32, isOutput=False)
    bv_ext = nc.declare_dram_parameter("bv", [1, D], BF16, isOutput=False)
    mA_ext = nc.declare_dram_parameter("maskA", [128, 4 * 512], BF16, isOutput=False)
    mB_ext = nc.declare_dram_parameter("maskB", [128, 4 * 512], BF16, isOutput=False)
    # per-slot 0/1 block-selection scalars: col i -> block0?, col 17+i -> block1?
    selq_ext = nc.declare_dram_parameter("selq", [128, 2 * SLOTS], F32, isOutput=False)
    # r-major (r*17+slot) selection masks for the row-sum split
    s0m_ext = nc.declare_dram_parameter("s0m", [128, 4 * SLOTS], F32, isOutput=False)
    s1m_ext = nc.declare_dram_parameter("s1m", [128, 4 * SLOTS], F32, isOutput=False)
    attn_ext = nc.declare_dram_parameter("attn", [SLOTS, 512, 512], BF16,
                                         isOutput=True)
    out_ext = nc.declare_dram_parameter("out", [2 * BLK, D], F32, isOutput=True)

    exps_dram = nc.dram_tensor("exps", [SLOTS, 512, 512], BF16)

    xTs_r = xTs_ext.rearrange("(kc p) s -> p kc s", p=128)    # [128,4,8704]
    xqT_r = xqT_ext.rearrange("(kc p) s -> p kc s", p=128)    # [128,4,1024]
    wq_r = wqT_ext.rearrange("(kc p) o -> p kc o", p=128)     # [128,4,512]
    wk_r = wkT_ext.rearrange("(kc p) o -> p kc o", p=128)
    wv_r = wvT_ext.rearrange("(kc p) o -> p kc o", p=128)
    out_r = out_ext.rearrange("(rg p) d -> p rg d", p=128)    # [128,8,512]

    with tile.TileContext(nc) as tc:
        with (
            tc.tile_pool(name="big", bufs=1) as big,
            tc.tile_pool(name="const", bufs=1) as const,
            tc.tile_pool(name="xt", bufs=2) as xtp,
            tc.tile_pool(name="ea", bufs=2) as eap,
            tc.tile_pool(name="eb", bufs=2) as ebp,
            tc.tile_pool(name="cur", bufs=2) as curp,
            tc.tile_pool(name="ps", bufs=2, space="PSUM") as psp,
        ):
            # ---- persistent SBUF ----
            kT = big.tile([128, 4, KW], BF16, tag="kT")         # 68 KB/part
            vv = big.tile([128, SLOTS * 4 * 512], BF16, tag="v")  # 68 KB/part
            qT = big.tile([128, 4, 2 * BLK], BF16, tag="qT")    # 8 KB/part
            pvacc = big.tile([128, 2 * 4 * 512], F32, tag="pv")  # 16 KB/part
            lpart = big.tile([128, 4 * SLOTS], F32, tag="lp")   # r-major
            lsum = big.tile([128, 8], F32, tag="ls")
            rl = big.tile([128, 8], F32, tag="rl")
            rsel = big.tile([128, 4 * SLOTS], F32, tag="rsel")
            tsum = big.tile([128, 4 * SLOTS], F32, tag="tsum")

            bqk = const.tile([128, 8], F32, tag="bqk")
            bv = const.tile([1, D], BF16, tag="bv")
            ones = const.tile([1, 128], BF16, tag="ones")
            mA = const.tile([128, 4, 512], BF16, tag="mA")
            mB = const.tile([128, 4, 512], BF16, tag="mB")
            selq = const.tile([128, 2 * SLOTS], F32, tag="selq")
            s0m = const.tile([128, 4 * SLOTS], F32, tag="s0m")
            s1m = const.tile([128, 4 * SLOTS], F32, tag="s1m")

            nc.sync.dma_start(bqk[:], bqk_ext[:])
            nc.sync.dma_start(bv[:], bv_ext[:])
            nc.sync.dma_start(mA[:], mA_ext.rearrange("p (r n) -> p r n", r=4))
            nc.sync.dma_start(mB[:], mB_ext.rearrange("p (r n) -> p r n", r=4))
            nc.sync.dma_start(selq[:], selq_ext[:])
            nc.sync.dma_start(s0m[:], s0m_ext[:])
            nc.sync.dma_start(s1m[:], s1m_ext[:])
            nc.gpsimd.memset(ones[:], 1.0)
            nc.gpsimd.memset(lpart[:], 0.0)
            nc.gpsimd.memset(pvacc[:], 0.0)

            # ---- projections ----
            wq = ebp.tile([128, 4, 512], BF16, tag="eb")
            wk = ebp.tile([128, 4, 512], BF16, tag="eb")
            wv = curp.tile([128, 4, 512], BF16, tag="qcur")
            nc.sync.dma_start(wq[:], wq_r[:])
            nc.sync.dma_start(wk[:], wk_r[:])
            nc.sync.dma_start(wv[:], wv_r[:])

            # qT = Wq @ xqT + bq -> [128(d), 4dc, 1024]
            for half in range(2):
                xqh = xtp.tile([128, 4, 512], BF16, tag="xt")
                nc.sync.dma_start(xqh[:], xqT_r[:, :, half * 512:(half + 1) * 512])
                psq = psp.tile([128, 2048], F32, tag="ps")
                for dc in range(4):
                    for kc in range(4):
                        nc.tensor.matmul(
                            psq[:, dc * 512:(dc + 1) * 512],
                            wq[:, kc, dc * 128:(dc + 1) * 128],
                            xqh[:, kc, :],
                            start=(kc == 0), stop=(kc == 3))
                for dc in range(4):
                    nc.scalar.activation(
                        qT[:, dc, half * 512:(half + 1) * 512],
                        psq[:, dc * 512:(dc + 1) * 512],
                        AF.Identity, bias=bqk[:, dc:dc + 1])

            # kT = Wk @ xTs + bk ; v = xs @ Wv.T + bv, over 17 permuted tiles
            for t in range(SLOTS):
                xt = xtp.tile([128, 4, 512], BF16, tag="xt")
                nc.sync.dma_start(xt[:], xTs_r[:, :, t * 512:(t + 1) * 512])

                psk = psp.tile([128, 2048], F32, tag="ps")
                for dc in range(4):
                    for kc in range(4):
                        nc.tensor.matmul(
                            psk[:, dc * 512:(dc + 1) * 512],
                            wk[:, kc, dc * 128:(dc + 1) * 128],
                            xt[:, kc, :],
                            start=(kc == 0), stop=(kc == 3))
                for dc in range(4):
                    nc.scalar.activation(
                        kT[:, dc, t * 512:(t + 1) * 512],
                        psk[:, dc * 512:(dc + 1) * 512],
                        AF.Identity, bias=bqk[:, 4 + dc:5 + dc])

                psv = psp.tile([128, 2048], F32, tag="ps")
                for sub in range(4):
                    for kc in range(4):
                        nc.tensor.matmul(
                            psv[:, sub * 512:(sub + 1) * 512],
                            xt[:, kc, sub * 128:(sub + 1) * 128],
                            wv[:, kc, :],
                            start=(kc == 0), stop=False)
                    nc.tensor.matmul(
                        psv[:, sub * 512:(sub + 1) * 512], ones[:, 0:128],
                        bv[:, :], start=False, stop=True)
                for sub in range(4):
                    nc.vector.tensor_copy(
                        vv[:, (t * 4 + sub) * 512:(t * 4 + sub + 1) * 512],
                        psv[:, sub * 512:(sub + 1) * 512])

            # ---- slot loop (all static; block routing via 0/1 scalars) ----
            for i in range(SLOTS):
                diag = i < 2
                # qcur = qT[block0]*s0 + qT[block1]*s1
                qcur = curp.tile([128, 4, 512], BF16, tag="qcur")
                nc.vector.tensor_scalar(qcur[:], qT[:, :, 0:512],
                                        selq[:, i:i + 1], None, ALU.mult)
                nc.vector.scalar_tensor_tensor(
                    qcur[:], qT[:, :, 512:1024], selq[:, SLOTS + i:SLOTS + i + 1],
                    qcur[:], ALU.mult, ALU.add)

                # scores A [sq, sk]
                psA = psp.tile([128, 2048], F32, tag="ps")
                for r in range(4):
                    for kc in range(4):
                        nc.tensor.matmul(
                            psA[:, r * 512:(r + 1) * 512],
                            qcur[:, kc, r * 128:(r + 1) * 128],
                            kT[:, kc, i * 512:(i + 1) * 512],
                            start=(kc == 0), stop=(kc == 3))
                if diag:
                    for r in range(4):
                        nc.vector.tensor_add(psA[:, r * 512:(r + 1) * 512],
                                             psA[:, r * 512:(r + 1) * 512],
                                             mA[:, r, :])
                ea = eap.tile([128, 4, 512], BF16, tag="ea")
                for r in range(4):
                    nc.scalar.activation(
                        ea[:, r, :], psA[:, r * 512:(r + 1) * 512], AF.Exp,
                        scale=float(SCALE),
                        accum_out=lpart[:, r * SLOTS + i:r * SLOTS + i + 1])
                nc.sync.dma_start(
                    exps_dram[i].rearrange("(r p) n -> p r n", p=128), ea[:])

                # scores B [sk, sq]
                psB = psp.tile([128, 2048], F32, tag="ps")
                for r in range(4):
                    for kc in range(4):
                        nc.tensor.matmul(
                            psB[:, r * 512:(r + 1) * 512],
                            kT[:, kc, i * 512 + r * 128:i * 512 + (r + 1) * 128],
                            qcur[:, kc, :],
                            start=(kc == 0), stop=(kc == 3))
                if diag:
                    for r in range(4):
                        nc.vector.tensor_add(psB[:, r * 512:(r + 1) * 512],
                                             psB[:, r * 512:(r + 1) * 512],
                                             mB[:, r, :])
                eb = ebp.tile([128, 4, 512], BF16, tag="eb")
                nc.scalar.activation(eb[:], psB.rearrange("p (r n) -> p r n", r=4),
                                     AF.Exp, scale=float(SCALE))

                # PV: psPV[sq, d] = sum_sub expT[sub].T @ v[sub]
                psPV = psp.tile([128, 2048], F32, tag="ps")
                for r in range(4):
                    for sub in range(4):
                        nc.tensor.matmul(
                            psPV[:, r * 512:(r + 1) * 512],
                            eb[:, sub, r * 128:(r + 1) * 128],
                            vv[:, (i * 4 + sub) * 512:(i * 4 + sub + 1) * 512],
                            start=(sub == 0), stop=(sub == 3))
                # route into the active block's accumulator
                nc.vector.scalar_tensor_tensor(
                    pvacc[:, 0:2048], psPV[:], selq[:, i:i + 1],
                    pvacc[:, 0:2048], ALU.mult, ALU.add)
                nc.vector.scalar_tensor_tensor(
                    pvacc[:, 2048:4096], psPV[:], selq[:, SLOTS + i:SLOTS + i + 1],
                    pvacc[:, 2048:4096], ALU.mult, ALU.add)

            # ---- softmax sums, reciprocal, output ----
            nc.vector.tensor_mul(tsum[:], lpart[:], s0m[:])
            for r in range(4):
                nc.vector.reduce_sum(lsum[:, r:r + 1],
                                     tsum[:, r * SLOTS:(r + 1) * SLOTS],
                                     axis=mybir.AxisListType.X)
            nc.vector.tensor_mul(tsum[:], lpart[:], s1m[:])
            for r in range(4):
                nc.vector.reduce_sum(lsum[:, 4 + r:5 + r],
                                     tsum[:, r * SLOTS:(r + 1) * SLOTS],
                                     axis=mybir.AxisListType.X)
            nc.vector.reciprocal(rl[:, 0:8], lsum[:, 0:8])

            # per-slot normalization scalars: rsel[:, r*17+i] = 1/l of (block_i, r)
            for r in range(4):
                nc.vector.tensor_scalar(rsel[:, r * SLOTS:(r + 1) * SLOTS],
                                        s0m[:, r * SLOTS:(r + 1) * SLOTS],
                                        rl[:, r:r + 1], None, ALU.mult)
                nc.vector.scalar_tensor_tensor(
                    rsel[:, r * SLOTS:(r + 1) * SLOTS],
                    s1m[:, r * SLOTS:(r + 1) * SLOTS], rl[:, 4 + r:5 + r],
                    rsel[:, r * SLOTS:(r + 1) * SLOTS], ALU.mult, ALU.add)

            for rg in range(8):
                nc.vector.tensor_scalar(
                    pvacc[:, rg * 512:(rg + 1) * 512],
                    pvacc[:, rg * 512:(rg + 1) * 512],
                    rl[:, rg:rg + 1], None, ALU.mult)
            nc.sync.dma_start(out_r[:], pvacc.rearrange("p (rg d) -> p rg d", rg=8))

            # ---- normalize pass: attn = exp * 1/l ----
            for i in range(SLOTS):
                st = eap.tile([128, 4, 512], BF16, tag="ea")
                nc.sync.dma_start(
                    st[:], exps_dram[i].rearrange("(r p) n -> p r n", p=128))
                for r in range(4):
                    nc.vector.tensor_scalar(
                        st[:, r, :], st[:, r, :],
                        rsel[:, r * SLOTS + i:r * SLOTS + i + 1], None, ALU.mult)
                nc.sync.dma_start(
                    attn_ext[i].rearrange("(r p) n -> p r n", p=128), st[:])

    split_waits(nc)
    return nc


@functools.lru_cache(maxsize=1)
def _get_nc():
    return build()


def _masks():
    i = np.arange(128)[:, None]
    j = np.arange(512)[None, :]
    bf = ml_dtypes.bfloat16
    mA = np.zeros((128, 4, 512), np.float32)
    mB = np.zeros((128, 4, 512), np.float32)
    for r in range(4):
        mA[:, r, :] = np.where(j > 128 * r + i, NEG, 0.0)
        mB[:, r, :] = np.where(128 * r + i > j, NEG, 0.0)
    return mA.reshape(128, -1).astype(bf), mB.reshape(128, -1).astype(bf)


def kernel(x, Wq, bq, Wk, bk, Wv, bv):
    x = np.asarray(x, np.float32)
    Wq = np.asarray(Wq, np.float32); bq = np.asarray(bq, np.float32)
    Wk = np.asarray(Wk, np.float32); bk = np.asarray(bk, np.float32)
    Wv = np.asarray(Wv, np.float32); bv = np.asarray(bv, np.float32)

    bf = ml_dtypes.bfloat16
    xT = np.ascontiguousarray(x.T).astype(bf)          # [512, 8192]
    wqT = np.ascontiguousarray(Wq.T).astype(bf)
    wkT = np.ascontiguousarray(Wk.T).astype(bf)
    wvT = np.ascontiguousarray(Wv.T).astype(bf)
    bqk = np.concatenate([bq.reshape(4, 128).T, bk.reshape(4, 128).T],
                         axis=1).astype(np.float32)
    bv_in = np.ascontiguousarray(bv[None, :]).astype(bf)
    mA, mB = _masks()

    in_maps = []
    for c in range(NCORES):
        slots = core_slots(c)
        b0, b1 = core_blocks(c)
        rows = np.r_[b0 * BLK:(b0 + 1) * BLK, b1 * BLK:(b1 + 1) * BLK]
        xqT = np.ascontiguousarray(x[rows].T).astype(bf)
        xTs = np.empty((D, KW), bf)
        for i, (_, t) in enumerate(slots):
            xTs[:, i * 512:(i + 1) * 512] = xT[:, t * 512:(t + 1) * 512]
        sel = np.zeros((128, 2 * SLOTS), np.float32)
        s0m = np.zeros((128, 4 * SLOTS), np.float32)
        s1m = np.zeros((128, 4 * SLOTS), np.float32)
        for i, (b, _) in enumerate(slots):
            sel[:, b * SLOTS + i] = 1.0
            for r in range(4):
                (s0m if b == 0 else s1m)[:, r * SLOTS + i] = 1.0
        in_maps.append({
            "xTs": xTs, "xqT": xqT, "wqT": wqT, "wkT": wkT, "wvT": wvT,
            "bqk": bqk, "bv": bv_in, "maskA": mA, "maskB": mB,
            "selq": sel, "s0m": s0m, "s1m": s1m,
        })

    nc = _get_nc()
    res = run_bass_kernel_spmd(nc, in_maps, core_ids=list(range(NCORES)))

    attn_full = np.zeros((S, S), np.float32)
    out_full = np.empty((S, D), np.float32)
    for c in range(NCORES):
        a = np.asarray(res.results[c]["attn"])
        o = np.asarray(res.results[c]["out"])
        b0, b1 = core_blocks(c)
        out_full[b0 * BLK:(b0 + 1) * BLK] = o[:BLK]
        out_full[b1 * BLK:(b1 + 1) * BLK] = o[BLK:]
        for i, (bsel, t) in enumerate(core_slots(c)):
            j = (b0, b1)[bsel]
            attn_full[j * BLK:(j + 1) * BLK, t * 512:(t + 1) * 512] = a[i]
    return out_full, attn_full
